# revision 26
# baseline (speedup 1.0000x reference)
"""Multi-head attention (qk-norm variant) on 8 TRN2 NeuronCores.

Sharding (Megatron-style, per spec hint): core c handles batch b=c//4 and
head-group hg=c%4 (4 of 16 heads). QKV is column-parallel (each core owns its
heads' rows of w_qkv), attention is fully local per (b, head), and the output
projection is row-parallel: each core produces a partial [N, DIM] f32 output
which the host sums per batch (the "unshard" step) and adds b_proj.

Per-core kernel (bf16 compute, fp32 PSUM accumulation):
  - x arrives pre-transposed (xT [DIM, N]) so the feature dim lies on SBUF
    partitions for all matmuls.
  - q,k are produced head-major ([d, tok], 2 heads stacked per 128
    partitions) for the scores matmul; layernorm over head_dim (the
    partition dim) is done with matmuls: centering via C2 = blockdiag(I-J/64)
    and per-row sum-of-squares via ones-column matmuls; the per-token rstd
    rows are broadcast across partitions with gpsimd.partition_broadcast
    (base-0 tiles; the consuming DVE muls use a PSUM first operand where
    partition bases differ, since walrus only requires equal bases when both
    TensorTensor inputs are SBUF).
  - softmax needs no max-subtraction: after qk-norm, |q|=|k|=8 exactly, so
    scores are in [-8, 8] and exp() is safe.
  - v is token-major with a fused ones-column, so the PV matmul produces the
    softmax denominator as psum row 64 for free; normalization is a rank-1
    scale applied after PV (reciprocal_approx_fast + partition_broadcast).
  - emission order interleaves the layernorm chains with qkv/attention
    matmul bursts so the PE never idles long enough to re-throttle (HAM).
"""
import numpy as np
import ml_dtypes

import concourse.bass as bass
import concourse.bacc as bacc
import concourse.tile as tile
from concourse import mybir
from concourse.bass_utils import run_bass_kernel_spmd

F32 = mybir.dt.float32
BF16 = mybir.dt.bfloat16
AF = mybir.ActivationFunctionType

B, N, DIM = 2, 2048, 1024
H, D = 16, 64
EPS = 1e-5
N_CORES = 8
HPC = 4              # heads per core
HF = HPC * D         # 256 local head features
KT = DIM // 128      # 8 contraction tiles
NT = N // 128        # 16 token tiles
NCH = N // 512       # 4 token chunks
SCALE = D ** -0.5

# set by test harness to request NTFF profiling
TRACE = False
LAST_EXEC_NS = None
LAST_RESULTS = None

_BUILD_CACHE = {}


def _build(has_qgamma, has_kgamma, has_qbeta, has_kbeta, has_vbias):
    key = (has_qgamma, has_kgamma, has_qbeta, has_kbeta, has_vbias)
    if key in _BUILD_CACHE:
        return _BUILD_CACHE[key]

    nc = bacc.Bacc("TRN2", target_bir_lowering=False, debug=False,
                   num_devices=N_CORES)

    xT_d = nc.dram_tensor("xT", [DIM, N], BF16, kind="ExternalInput")
    wqkT_d = nc.dram_tensor("wqkT", [DIM, 2 * HF], BF16, kind="ExternalInput")
    wvT_d = nc.dram_tensor("wvT", [DIM, HF], BF16, kind="ExternalInput")
    wpT_d = nc.dram_tensor("wpT", [HF, DIM], BF16, kind="ExternalInput")
    bqk_d = nc.dram_tensor("bqk_cols", [128, 4], F32, kind="ExternalInput")
    bvT_d = nc.dram_tensor("bvT", [1, HF], BF16, kind="ExternalInput")
    O2_d = nc.dram_tensor("O2", [128, 2], BF16, kind="ExternalInput")
    ones_d = nc.dram_tensor("ones512", [1, 512], BF16, kind="ExternalInput")
    gamma_d = beta_d = None
    if has_qgamma or has_kgamma:
        gamma_d = nc.dram_tensor("gamma_cols", [128, 2], F32, kind="ExternalInput")
    if has_qbeta or has_kbeta:
        beta_d = nc.dram_tensor("beta_cols", [128, 2], F32, kind="ExternalInput")
    out_d = nc.dram_tensor("out_partial", [N, DIM], BF16, kind="ExternalOutput")

    with tile.TileContext(nc) as tc:
        with (
            tc.tile_pool(name="persist", bufs=1) as pp,
            tc.tile_pool(name="work", bufs=2) as wp,
            tc.tile_pool(name="psum", bufs=1, space="PSUM") as psp,
            tc.tile_pool(name="dram", bufs=3, space="DRAM") as dp,
        ):
            # ---- persistent SBUF tensors ----
            xT = [pp.tile([128, N], BF16, name=f"xT{i}") for i in range(KT)]
            wqk = [pp.tile([128, 2 * HF], BF16, name=f"wqk{i}") for i in range(KT)]
            wv = [pp.tile([128, HF], BF16, name=f"wv{i}") for i in range(KT)]
            wpj = [pp.tile([128, DIM], BF16, name=f"wpj{i}") for i in range(2)]
            O2 = pp.tile([128, 2], BF16)
            ones512 = pp.tile([1, 512], BF16)
            bqk = pp.tile([128, 4], F32)
            bvT = pp.tile([1, HF], BF16)
            eps_sb = pp.tile([2, 1], F32)
            gamma_c = pp.tile([128, 2], F32) if gamma_d is not None else None
            beta_c = pp.tile([128, 2], F32) if beta_d is not None else None

            # v token-major with a ones column at index 64 (width 66 keeps the
            # innermost dim even for DVE perf modes)
            v_sb = pp.tile([128, NT, HPC, 66], BF16)
            # qk: raw -> centered -> normalized, all in place
            # [., g, tok] with g in {q01, q23, k01, k23}
            qkt = pp.tile([128, 4, N], BF16)
            outT_n = pp.tile([128, 2, N], BF16)   # attn out, head-major

            for i in range(KT):
                nc.sync.dma_start(out=xT[i], in_=xT_d.ap()[i * 128:(i + 1) * 128, :])
                nc.sync.dma_start(out=wqk[i], in_=wqkT_d.ap()[i * 128:(i + 1) * 128, :])
                nc.sync.dma_start(out=wv[i], in_=wvT_d.ap()[i * 128:(i + 1) * 128, :])
            for i in range(2):
                nc.sync.dma_start(out=wpj[i], in_=wpT_d.ap()[i * 128:(i + 1) * 128, :])
            for t, d in [(O2, O2_d), (ones512, ones_d),
                         (bqk, bqk_d), (bvT, bvT_d)]:
                nc.sync.dma_start(out=t, in_=d.ap())
            if gamma_c is not None:
                nc.sync.dma_start(out=gamma_c, in_=gamma_d.ap())
            if beta_c is not None:
                nc.sync.dma_start(out=beta_c, in_=beta_d.ap())

            nc.vector.memset(eps_sb, EPS)
            nc.vector.memset(v_sb[:, :, :, 64:66], 0.0)
            nc.vector.memset(v_sb[:, :, :, 64:65], 1.0)

            def qk_feats(mt, ch):
                """q/k head-major projection for feature tile mt, chunk ch."""
                csl = slice(ch * 512, (ch + 1) * 512)
                ps_qk = psp.tile([128, 512], F32, tag="misc", bufs=2,
                                 name="ps_qk")
                for kt in range(KT):
                    nc.tensor.matmul(
                        ps_qk,
                        wqk[kt][:, mt * 128:(mt + 1) * 128],
                        xT[kt][:, csl],
                        start=(kt == 0), stop=(kt == KT - 1))
                # fold the qkv bias (per-feature = per-partition here)
                nc.vector.tensor_scalar_add(
                    qkt[:, mt, csl], ps_qk, bqk[:, mt:mt + 1])

            def v_feats(tt):
                """v token-major projection for token tile tt."""
                tsl = slice(tt * 128, (tt + 1) * 128)
                ps_v = psp.tile([128, 512], F32, tag="misc", bufs=2,
                                name="ps_v")
                for kt in range(KT):
                    nc.tensor.matmul(
                        ps_v[:, 0:HF], xT[kt][:, tsl], wv[kt],
                        start=(kt == 0),
                        stop=(not has_vbias and kt == KT - 1))
                if has_vbias:
                    nc.tensor.matmul(ps_v[:, 0:HF], ones512[:, 0:128],
                                     bvT, start=False, stop=True)
                nc.vector.tensor_copy(
                    v_sb[:, tt, :, 0:64],
                    ps_v[:, 0:HF].rearrange("p (h d) -> p h d", h=HPC))

            def ln_chunk(g, ch):
                """layernorm (over partition-axis head_dim) for group g,
                token chunk ch. The qkv weights are pre-centered on the host
                (centering is linear), so qkt already holds centered values;
                this computes rstd and applies it (gamma is folded into B2)."""
                is_q = g < 2
                gcol = None
                if is_q and has_qgamma:
                    gcol = gamma_c[:, 0:1]
                elif not is_q and has_kgamma:
                    gcol = gamma_c[:, 1:2]
                bcol = None
                if is_q and has_qbeta:
                    bcol = beta_c[:, 0:1]
                elif not is_q and has_kbeta:
                    bcol = beta_c[:, 1:2]

                csl = slice(ch * 512, (ch + 1) * 512)
                sq = wp.tile([128, 512], BF16, tag="sq", bufs=3)
                nc.vector.tensor_mul(sq, qkt[:, g, csl], qkt[:, g, csl])
                ps_ssq = psp.tile([2, 512], F32, tag="misc", bufs=2,
                                  name="ps_ssq")
                nc.tensor.matmul(ps_ssq, O2, sq, start=True, stop=True)
                std = wp.tile([2, 512], F32, tag="std", bufs=3)
                nc.scalar.activation(std, ps_ssq, AF.Sqrt,
                                     scale=1.0 / D, bias=eps_sb)
                rstd = wp.tile([2, 512], F32, tag="rstd", bufs=3)
                nc.vector.reciprocal_approx_fast(rstd, std)
                # broadcast the rstd rows across their 64-partition halves
                # through a DRAM bounce (DRAM sources may repeat partitions);
                # both qn muls are then same-base SBUF ops, no PE involved
                dr = dp.tile([2, 512], F32, name="dr")
                nc.sync.dma_start(out=dr, in_=rstd)
                rb = wp.tile([128, 512], F32, tag="rb", bufs=3)
                for j in range(2):
                    row = dr[j:j + 1, :]
                    bc = bass.AP(tensor=row.tensor, offset=row.offset,
                                 ap=[[0, 64]] + list(row.ap[1:]))
                    nc.sync.dma_start(out=rb[64 * j:64 * (j + 1), :], in_=bc)
                nc.vector.tensor_mul(qkt[0:64, g, csl], qkt[0:64, g, csl],
                                     rb[0:64, :])
                nc.vector.tensor_mul(qkt[64:128, g, csl], qkt[64:128, g, csl],
                                     rb[64:128, :])
                if gcol is not None:
                    nc.vector.tensor_scalar_mul(qkt[:, g, csl],
                                                qkt[:, g, csl], gcol)
                if bcol is not None:
                    nc.vector.tensor_scalar_add(qkt[:, g, csl],
                                                qkt[:, g, csl], bcol)

            def attn_scores(gq, qc_i):
                """scores+exp for head pair gq, query chunk; returns the
                paired exp tile. Both heads write one 2-bank psum tile:
                their matmuls sit at base partitions 0/64 (disjoint PE row
                groups) with no slot boundary between them, so the PE runs
                them concurrently; one FD=1024 exp serves both heads."""
                qsl = slice(qc_i * 512, (qc_i + 1) * 512)
                exp_pair = wp.tile([128, NT, 2, 512], BF16, tag="exp",
                                   bufs=2, name="exp_pair")
                for kt in range(NT):
                    ktsl = slice(kt * 128, (kt + 1) * 128)
                    ps_s = psp.tile([128, 2, 512], F32, tag="score",
                                    bufs=2, name="ps_s")
                    for hp in range(2):
                        p0 = hp * 64
                        nc.tensor.matmul(ps_s[:, hp, :],
                                         qkt[p0:p0 + 64, 2 + gq, ktsl],
                                         qkt[p0:p0 + 64, gq, qsl],
                                         start=True, stop=True)
                    nc.scalar.activation(exp_pair[:, kt, :, :], ps_s,
                                         AF.Exp, scale=SCALE)
                return exp_pair

            def attn_pv(gq, qc_i, exp_pair):
                """PV+normalize for head pair gq, query chunk."""
                qsl = slice(qc_i * 512, (qc_i + 1) * 512)
                for hp in range(2):
                    p0 = hp * 64
                    h = 2 * gq + hp
                    ps_o = psp.tile([65, 512], F32, tag="pvc", bufs=2,
                                    name="ps_o")
                    for kt in range(NT):
                        nc.tensor.matmul(ps_o, v_sb[:, kt, h, 0:65],
                                         exp_pair[:, kt, hp, :],
                                         start=(kt == 0), stop=(kt == NT - 1))
                    # reciprocal_approx_fast misreads PSUM sources — stage
                    # the denominator row through SBUF first
                    den = wp.tile([1, 512], F32, tag="den", bufs=3)
                    nc.vector.tensor_copy(den, ps_o[64:65, :])
                    rec = wp.tile([1, 512], F32, tag="rec", bufs=3)
                    nc.vector.reciprocal_approx_fast(rec, den)
                    rb2 = wp.tile([64, 512], F32, tag="rb2", bufs=3)
                    nc.gpsimd.partition_broadcast(rb2, rec)
                    nc.vector.tensor_mul(outT_n[p0:p0 + 64, gq, qsl],
                                         ps_o[0:64, :], rb2)

            def proj_chunk(qc_i):
                """output projection for this chunk's 4 token tiles."""
                for tt in range(qc_i * 4, qc_i * 4 + 4):
                    tsl = slice(tt * 128, (tt + 1) * 128)
                    for fn in range(2):
                        fsl = slice(fn * 512, (fn + 1) * 512)
                        ps_p = psp.tile([128, 512], F32, tag="misc", bufs=2,
                                        name="ps_p")
                        for t in range(2):
                            nc.tensor.matmul(ps_p, outT_n[:, t, tsl],
                                             wpj[t][:, fsl],
                                             start=(t == 0), stop=(t == 1))
                        ostg = wp.tile([128, 512], BF16, tag="ostg", bufs=3)
                        nc.vector.tensor_copy(ostg, ps_p)
                        nc.sync.dma_start(out=out_d.ap()[tsl, fsl], in_=ostg)

            # ---- emission: interleave LN chains with matmul bursts ----
            with nc.named_scope("qkv01"):
                for ch in range(NCH):
                    qk_feats(0, ch)      # q01
                    qk_feats(2, ch)      # k01
                    if ch > 0:
                        ln_chunk(0, ch - 1)
                        ln_chunk(2, ch - 1)
            with nc.named_scope("ln01_qkv23"):
                ln_chunk(0, NCH - 1)
                ln_chunk(2, NCH - 1)
                for ch in range(NCH):
                    qk_feats(1, ch)      # q23 (fills PE while LN01 chains run)
                for ch in range(NCH):
                    qk_feats(3, ch)      # k23
            with nc.named_scope("attn_pipe"):
                for ch in range(NCH):
                    ln_chunk(1, ch)
                    ln_chunk(3, ch)
                units = [(gq, qc) for gq in range(2) for qc in range(NCH)]
                pending = None   # (gq, qc, exp_pair) awaiting PV
                for ui, u in enumerate(units):
                    gq, qc_i = u
                    qsl = slice(qc_i * 512, (qc_i + 1) * 512)
                    exp_pair = wp.tile([128, NT, 2, 512], BF16, tag="exp",
                                       bufs=2, name="exp_pair")
                    # previous unit's PV accumulators, zippered per k-tile
                    # with this unit's score pairs so the PE keeps feeding
                    # ACT new score tiles even while PV matmuls run
                    ps_os = None
                    if pending is not None:
                        ps_os = [psp.tile([65, 512], F32, tag="pvc", bufs=2,
                                          name=f"ps_o{hp}") for hp in range(2)]
                    for kt in range(NT):
                        ktsl = slice(kt * 128, (kt + 1) * 128)
                        ps_s = psp.tile([128, 2, 512], F32, tag="score",
                                        bufs=2, name="ps_s")
                        for hp in range(2):
                            p0 = hp * 64
                            nc.tensor.matmul(ps_s[:, hp, :],
                                             qkt[p0:p0 + 64, 2 + gq, ktsl],
                                             qkt[p0:p0 + 64, gq, qsl],
                                             start=True, stop=True)
                        nc.scalar.activation(exp_pair[:, kt, :, :], ps_s,
                                             AF.Exp, scale=SCALE)
                        if ps_os is not None:
                            pg, pq, pep = pending
                            for hp in range(2):
                                nc.tensor.matmul(
                                    ps_os[hp], v_sb[:, kt, 2 * pg + hp, 0:65],
                                    pep[:, kt, hp, :],
                                    start=(kt == 0), stop=(kt == NT - 1))
                    if ui == 0:
                        # v projections only gate the first PV
                        for tt in range(NT):
                            v_feats(tt)
                    if pending is not None:
                        pg, pq, pep = pending
                        psl = slice(pq * 512, (pq + 1) * 512)
                        for hp in range(2):
                            p0 = hp * 64
                            den = wp.tile([1, 512], F32, tag="den", bufs=3)
                            nc.vector.tensor_copy(den, ps_os[hp][64:65, :])
                            rec = wp.tile([1, 512], F32, tag="rec", bufs=3)
                            nc.vector.reciprocal_approx_fast(rec, den)
                            rb2 = wp.tile([64, 512], F32, tag="rb2", bufs=3)
                            nc.gpsimd.partition_broadcast(rb2, rec)
                            nc.vector.tensor_mul(outT_n[p0:p0 + 64, pg, psl],
                                                 ps_os[hp][0:64, :], rb2)
                        if pg == 1:
                            proj_chunk(pq)
                    pending = (gq, qc_i, exp_pair)
                attn_pv(*pending)
                proj_chunk(pending[1])

    nc.compile()
    _BUILD_CACHE[key] = nc
    return nc


def _bf16(a):
    return np.ascontiguousarray(a).astype(ml_dtypes.bfloat16)


def kernel(**inputs):
    global LAST_EXEC_NS
    x = np.asarray(inputs["x"], np.float32)
    w_qkv = np.asarray(inputs["w_qkv"], np.float32)
    b_qkv = np.asarray(inputs["b_qkv"], np.float32)
    q_gamma = np.asarray(inputs["q_gamma"], np.float32)
    q_beta = np.asarray(inputs["q_beta"], np.float32)
    k_gamma = np.asarray(inputs["k_gamma"], np.float32)
    k_beta = np.asarray(inputs["k_beta"], np.float32)
    w_proj = np.asarray(inputs["w_proj"], np.float32)
    b_proj = np.asarray(inputs["b_proj"], np.float32)

    has_qgamma = not bool(np.all(q_gamma == 1.0))
    has_kgamma = not bool(np.all(k_gamma == 1.0))
    has_qbeta = bool(np.any(q_beta != 0.0))
    has_kbeta = bool(np.any(k_beta != 0.0))
    has_vbias = bool(np.any(b_qkv[2 * DIM:3 * DIM] != 0.0))
    nc = _build(has_qgamma, has_kgamma, has_qbeta, has_kbeta, has_vbias)

    # shared constants
    Cd = np.eye(D, dtype=np.float32) - 1.0 / D   # centering matrix (folded
    O2 = np.zeros((128, 2), np.float32)          # into the qkv weights below)
    O2[:D, 0] = 1.0
    O2[D:, 1] = 1.0
    gamma_cols = np.stack([np.tile(q_gamma, 2), np.tile(k_gamma, 2)],
                          axis=1).astype(np.float32)
    ones512 = np.ones((1, 512), np.float32)
    beta_cols = np.stack([np.tile(q_beta, 2), np.tile(k_beta, 2)],
                         axis=1).astype(np.float32)
    # pre-center the q/k projection weights and biases per head:
    # LN(Wx+b) centering is linear, so fold (I - J/64) into W and b
    w_qkv = w_qkv.copy()
    b_qkv = b_qkv.copy()
    for h in range(2 * H):            # 16 q heads then 16 k heads
        rs = slice(h * D, (h + 1) * D)
        w_qkv[rs] = Cd @ w_qkv[rs]
        b_qkv[rs] = Cd @ b_qkv[rs]

    in_maps = []
    for c in range(N_CORES):
        b, hg = divmod(c, 4)
        rows = slice(hg * HF, (hg + 1) * HF)
        q_l = w_qkv[0 * DIM:1 * DIM][rows]           # [256, 1024]
        k_l = w_qkv[1 * DIM:2 * DIM][rows]
        v_l = w_qkv[2 * DIM:3 * DIM][rows]
        bq_l = b_qkv[0 * DIM:1 * DIM][rows]
        bk_l = b_qkv[1 * DIM:2 * DIM][rows]
        bv_l = b_qkv[2 * DIM:3 * DIM][rows]
        bqk_cols = np.stack([bq_l[:128], bq_l[128:], bk_l[:128], bk_l[128:]],
                            axis=1).astype(np.float32)
        m = {
            "xT": _bf16(x[b].T),                          # [1024, 2048]
            "wqkT": _bf16(np.concatenate([q_l, k_l], 0).T),   # [1024, 512]
            "wvT": _bf16(v_l.T),                          # [1024, 256]
            "wpT": _bf16(w_proj[:, rows].T),              # [256, 1024]
            "bqk_cols": bqk_cols,
            "bvT": _bf16(bv_l[None, :]),
            "O2": _bf16(O2),
            "ones512": _bf16(ones512),
        }
        if has_qgamma or has_kgamma:
            m["gamma_cols"] = gamma_cols
        if has_qbeta or has_kbeta:
            m["beta_cols"] = beta_cols
        in_maps.append(m)

    res = run_bass_kernel_spmd(nc, in_maps, core_ids=list(range(N_CORES)),
                               trace=TRACE)
    LAST_EXEC_NS = res.exec_time_ns
    globals()["LAST_RESULTS"] = res

    out = np.zeros((B, N, DIM), np.float32)
    for c in range(N_CORES):
        out[c // 4] += np.asarray(res.results[c]["out_partial"], np.float32)
    out += b_proj[None, None, :]
    return out


# revision 27
# speedup vs baseline: 1.0361x; 1.0361x over previous
"""Multi-head attention (qk-norm variant) on 8 TRN2 NeuronCores.

Sharding (Megatron-style, per spec hint): core c handles batch b=c//4 and
head-group hg=c%4 (4 of 16 heads). QKV is column-parallel (each core owns its
heads' rows of w_qkv), attention is fully local per (b, head), and the output
projection is row-parallel: each core produces a partial [N, DIM] f32 output
which the host sums per batch (the "unshard" step) and adds b_proj.

Per-core kernel (bf16 compute, fp32 PSUM accumulation):
  - x arrives pre-transposed (xT [DIM, N]) so the feature dim lies on SBUF
    partitions for all matmuls.
  - q,k are produced head-major ([d, tok], 2 heads stacked per 128
    partitions) for the scores matmul; layernorm over head_dim (the
    partition dim) is done with matmuls: centering via C2 = blockdiag(I-J/64)
    and per-row sum-of-squares via ones-column matmuls; the per-token rstd
    rows are broadcast across partitions with gpsimd.partition_broadcast
    (base-0 tiles; the consuming DVE muls use a PSUM first operand where
    partition bases differ, since walrus only requires equal bases when both
    TensorTensor inputs are SBUF).
  - softmax needs no max-subtraction: after qk-norm, |q|=|k|=8 exactly, so
    scores are in [-8, 8] and exp() is safe.
  - v is token-major with a fused ones-column, so the PV matmul produces the
    softmax denominator as psum row 64 for free; normalization is a rank-1
    scale applied after PV (reciprocal_approx_fast + partition_broadcast).
  - emission order interleaves the layernorm chains with qkv/attention
    matmul bursts so the PE never idles long enough to re-throttle (HAM).
"""
import numpy as np
import ml_dtypes

import concourse.bass as bass
import concourse.bacc as bacc
import concourse.tile as tile
from concourse import mybir
from concourse.bass_utils import run_bass_kernel_spmd

F32 = mybir.dt.float32
BF16 = mybir.dt.bfloat16
AF = mybir.ActivationFunctionType

B, N, DIM = 2, 2048, 1024
H, D = 16, 64
EPS = 1e-5
N_CORES = 8
HPC = 4              # heads per core
HF = HPC * D         # 256 local head features
KT = DIM // 128      # 8 contraction tiles
NT = N // 128        # 16 token tiles
NCH = N // 512       # 4 token chunks
SCALE = D ** -0.5

# set by test harness to request NTFF profiling
TRACE = False
LAST_EXEC_NS = None
LAST_RESULTS = None

_BUILD_CACHE = {}


def _build(has_qgamma, has_kgamma, has_qbeta, has_kbeta, has_vbias):
    key = (has_qgamma, has_kgamma, has_qbeta, has_kbeta, has_vbias)
    if key in _BUILD_CACHE:
        return _BUILD_CACHE[key]

    nc = bacc.Bacc("TRN2", target_bir_lowering=False, debug=False,
                   num_devices=N_CORES)

    xT_d = nc.dram_tensor("xT", [DIM, N], BF16, kind="ExternalInput")
    wqkT_d = nc.dram_tensor("wqkT", [DIM, 2 * HF], BF16, kind="ExternalInput")
    wvT_d = nc.dram_tensor("wvT", [DIM, HF], BF16, kind="ExternalInput")
    wpT_d = nc.dram_tensor("wpT", [HF, DIM], BF16, kind="ExternalInput")
    bqk_d = nc.dram_tensor("bqk_cols", [128, 4], F32, kind="ExternalInput")
    bvT_d = nc.dram_tensor("bvT", [1, HF], BF16, kind="ExternalInput")
    O2_d = nc.dram_tensor("O2", [128, 2], BF16, kind="ExternalInput")
    ones_d = nc.dram_tensor("ones512", [1, 512], BF16, kind="ExternalInput")
    gamma_d = beta_d = None
    if has_qgamma or has_kgamma:
        gamma_d = nc.dram_tensor("gamma_cols", [128, 2], F32, kind="ExternalInput")
    if has_qbeta or has_kbeta:
        beta_d = nc.dram_tensor("beta_cols", [128, 2], F32, kind="ExternalInput")
    out_d = nc.dram_tensor("out_partial", [N, DIM], BF16, kind="ExternalOutput")

    with tile.TileContext(nc) as tc:
        with (
            tc.tile_pool(name="persist", bufs=1) as pp,
            tc.tile_pool(name="work", bufs=2) as wp,
            tc.tile_pool(name="psum", bufs=1, space="PSUM") as psp,
            tc.tile_pool(name="dram", bufs=3, space="DRAM") as dp,
        ):
            # ---- persistent SBUF tensors ----
            xT = [pp.tile([128, N], BF16, name=f"xT{i}") for i in range(KT)]
            wqk = [pp.tile([128, 2 * HF], BF16, name=f"wqk{i}") for i in range(KT)]
            wv = [pp.tile([128, HF], BF16, name=f"wv{i}") for i in range(KT)]
            wpj = [pp.tile([128, DIM], BF16, name=f"wpj{i}") for i in range(2)]
            O2 = pp.tile([128, 2], BF16)
            ones512 = pp.tile([1, 512], BF16)
            bqk = pp.tile([128, 4], F32)
            bvT = pp.tile([1, HF], BF16)
            eps_sb = pp.tile([2, 1], F32)
            gamma_c = pp.tile([128, 2], F32) if gamma_d is not None else None
            beta_c = pp.tile([128, 2], F32) if beta_d is not None else None

            # v token-major with a ones column at index 64 (width 66 keeps the
            # innermost dim even for DVE perf modes)
            v_sb = pp.tile([128, NT, HPC, 66], BF16)
            # qk: raw -> centered -> normalized, all in place
            # [., g, tok] with g in {q01, q23, k01, k23}
            qkt = pp.tile([128, 4, N], BF16)
            outT_n = pp.tile([128, 2, N], BF16)   # attn out, head-major

            for i in range(KT):
                nc.sync.dma_start(out=xT[i], in_=xT_d.ap()[i * 128:(i + 1) * 128, :])
                nc.sync.dma_start(out=wqk[i], in_=wqkT_d.ap()[i * 128:(i + 1) * 128, :])
                nc.sync.dma_start(out=wv[i], in_=wvT_d.ap()[i * 128:(i + 1) * 128, :])
            for i in range(2):
                nc.sync.dma_start(out=wpj[i], in_=wpT_d.ap()[i * 128:(i + 1) * 128, :])
            for t, d in [(O2, O2_d), (ones512, ones_d),
                         (bqk, bqk_d), (bvT, bvT_d)]:
                nc.sync.dma_start(out=t, in_=d.ap())
            if gamma_c is not None:
                nc.sync.dma_start(out=gamma_c, in_=gamma_d.ap())
            if beta_c is not None:
                nc.sync.dma_start(out=beta_c, in_=beta_d.ap())

            nc.vector.memset(eps_sb, EPS)
            nc.vector.memset(v_sb[:, :, :, 64:66], 0.0)
            nc.vector.memset(v_sb[:, :, :, 64:65], 1.0)

            def qk_feats(mt, ch):
                """q/k head-major projection for feature tile mt, chunk ch."""
                csl = slice(ch * 512, (ch + 1) * 512)
                ps_qk = psp.tile([128, 512], F32, tag="misc", bufs=2,
                                 name="ps_qk")
                for kt in range(KT):
                    nc.tensor.matmul(
                        ps_qk,
                        wqk[kt][:, mt * 128:(mt + 1) * 128],
                        xT[kt][:, csl],
                        start=(kt == 0), stop=(kt == KT - 1))
                # fold the qkv bias (per-feature = per-partition here)
                nc.vector.tensor_scalar_add(
                    qkt[:, mt, csl], ps_qk, bqk[:, mt:mt + 1])

            def v_feats(tt):
                """v token-major projection for token tile tt."""
                tsl = slice(tt * 128, (tt + 1) * 128)
                ps_v = psp.tile([128, 512], F32, tag="misc", bufs=2,
                                name="ps_v")
                for kt in range(KT):
                    nc.tensor.matmul(
                        ps_v[:, 0:HF], xT[kt][:, tsl], wv[kt],
                        start=(kt == 0),
                        stop=(not has_vbias and kt == KT - 1))
                if has_vbias:
                    nc.tensor.matmul(ps_v[:, 0:HF], ones512[:, 0:128],
                                     bvT, start=False, stop=True)
                nc.vector.tensor_copy(
                    v_sb[:, tt, :, 0:64],
                    ps_v[:, 0:HF].rearrange("p (h d) -> p h d", h=HPC))

            def ln_chunk(g, ch):
                """layernorm (over partition-axis head_dim) for group g,
                token chunk ch. The qkv weights are pre-centered on the host
                (centering is linear), so qkt already holds centered values;
                this computes rstd and applies it (gamma is folded into B2)."""
                is_q = g < 2
                gcol = None
                if is_q and has_qgamma:
                    gcol = gamma_c[:, 0:1]
                elif not is_q and has_kgamma:
                    gcol = gamma_c[:, 1:2]
                bcol = None
                if is_q and has_qbeta:
                    bcol = beta_c[:, 0:1]
                elif not is_q and has_kbeta:
                    bcol = beta_c[:, 1:2]

                csl = slice(ch * 512, (ch + 1) * 512)
                sq = wp.tile([128, 512], BF16, tag="sq", bufs=3)
                nc.vector.tensor_mul(sq, qkt[:, g, csl], qkt[:, g, csl])
                ps_ssq = psp.tile([2, 512], F32, tag="misc", bufs=2,
                                  name="ps_ssq")
                nc.tensor.matmul(ps_ssq, O2, sq, start=True, stop=True)
                std = wp.tile([2, 512], F32, tag="std", bufs=3)
                nc.scalar.activation(std, ps_ssq, AF.Sqrt,
                                     scale=1.0 / D, bias=eps_sb)
                rstd = wp.tile([2, 512], F32, tag="rstd", bufs=3)
                nc.vector.reciprocal_approx_fast(rstd, std)
                # broadcast the rstd rows across their 64-partition halves
                # through a DRAM bounce (DRAM sources may repeat partitions);
                # both qn muls are then same-base SBUF ops, no PE involved
                dr = dp.tile([2, 512], F32, name="dr")
                nc.sync.dma_start(out=dr, in_=rstd)
                rb = wp.tile([128, 512], F32, tag="rb", bufs=3)
                for j in range(2):
                    row = dr[j:j + 1, :]
                    bc = bass.AP(tensor=row.tensor, offset=row.offset,
                                 ap=[[0, 64]] + list(row.ap[1:]))
                    nc.sync.dma_start(out=rb[64 * j:64 * (j + 1), :], in_=bc)
                nc.vector.tensor_mul(qkt[0:64, g, csl], qkt[0:64, g, csl],
                                     rb[0:64, :])
                nc.vector.tensor_mul(qkt[64:128, g, csl], qkt[64:128, g, csl],
                                     rb[64:128, :])
                if gcol is not None:
                    nc.vector.tensor_scalar_mul(qkt[:, g, csl],
                                                qkt[:, g, csl], gcol)
                if bcol is not None:
                    nc.vector.tensor_scalar_add(qkt[:, g, csl],
                                                qkt[:, g, csl], bcol)

            def attn_scores(gq, qc_i):
                """scores+exp for head pair gq, query chunk; returns the
                paired exp tile. Both heads write one 2-bank psum tile:
                their matmuls sit at base partitions 0/64 (disjoint PE row
                groups) with no slot boundary between them, so the PE runs
                them concurrently; one FD=1024 exp serves both heads."""
                qsl = slice(qc_i * 512, (qc_i + 1) * 512)
                exp_pair = wp.tile([128, NT, 2, 512], BF16, tag="exp",
                                   bufs=2, name="exp_pair")
                for kt in range(NT):
                    ktsl = slice(kt * 128, (kt + 1) * 128)
                    ps_s = psp.tile([128, 2, 512], F32, tag="score",
                                    bufs=2, name="ps_s")
                    for hp in range(2):
                        p0 = hp * 64
                        nc.tensor.matmul(ps_s[:, hp, :],
                                         qkt[p0:p0 + 64, 2 + gq, ktsl],
                                         qkt[p0:p0 + 64, gq, qsl],
                                         start=True, stop=True)
                    nc.scalar.activation(exp_pair[:, kt, :, :], ps_s,
                                         AF.Exp, scale=SCALE)
                return exp_pair

            def attn_pv(gq, qc_i, exp_pair):
                """PV+normalize for head pair gq, query chunk."""
                qsl = slice(qc_i * 512, (qc_i + 1) * 512)
                for hp in range(2):
                    p0 = hp * 64
                    h = 2 * gq + hp
                    ps_o = psp.tile([65, 512], F32, tag="pvc", bufs=2,
                                    name="ps_o")
                    for kt in range(NT):
                        nc.tensor.matmul(ps_o, v_sb[:, kt, h, 0:65],
                                         exp_pair[:, kt, hp, :],
                                         start=(kt == 0), stop=(kt == NT - 1))
                    # reciprocal_approx_fast misreads PSUM sources — stage
                    # the denominator row through SBUF first
                    den = wp.tile([1, 512], F32, tag="den", bufs=3)
                    nc.vector.tensor_copy(den, ps_o[64:65, :])
                    rec = wp.tile([1, 512], F32, tag="rec", bufs=3)
                    nc.vector.reciprocal_approx_fast(rec, den)
                    rb2 = wp.tile([64, 512], F32, tag="rb2", bufs=3)
                    nc.gpsimd.partition_broadcast(rb2, rec)
                    nc.vector.tensor_mul(outT_n[p0:p0 + 64, gq, qsl],
                                         ps_o[0:64, :], rb2)

            def proj_chunk(qc_i):
                """output projection for this chunk's 4 token tiles."""
                for tt in range(qc_i * 4, qc_i * 4 + 4):
                    tsl = slice(tt * 128, (tt + 1) * 128)
                    for fn in range(2):
                        fsl = slice(fn * 512, (fn + 1) * 512)
                        ps_p = psp.tile([128, 512], F32, tag="misc", bufs=2,
                                        name="ps_p")
                        for t in range(2):
                            nc.tensor.matmul(ps_p, outT_n[:, t, tsl],
                                             wpj[t][:, fsl],
                                             start=(t == 0), stop=(t == 1))
                        ostg = wp.tile([128, 512], BF16, tag="ostg", bufs=3)
                        nc.vector.tensor_copy(ostg, ps_p)
                        nc.sync.dma_start(out=out_d.ap()[tsl, fsl], in_=ostg)

            # ---- emission: interleave LN chains with matmul bursts ----
            with nc.named_scope("qkv01"):
                for ch in range(NCH):
                    qk_feats(0, ch)      # q01
                    qk_feats(2, ch)      # k01
                    if ch > 0:
                        ln_chunk(0, ch - 1)
                        ln_chunk(2, ch - 1)
            with nc.named_scope("ln01_qkv23"):
                ln_chunk(0, NCH - 1)
                ln_chunk(2, NCH - 1)
                for ch in range(NCH):
                    qk_feats(1, ch)      # q23 (fills PE while LN01 chains run)
                for ch in range(NCH):
                    qk_feats(3, ch)      # k23
            with nc.named_scope("attn_pipe"):
                for ch in range(NCH):
                    ln_chunk(1, ch)
                    ln_chunk(3, ch)
                units = [(gq, qc) for gq in range(2) for qc in range(NCH)]
                pending = None   # (gq, qc, exp_pair) awaiting PV
                for ui, u in enumerate(units):
                    ep = attn_scores(*u)
                    if ui == 0:
                        # v projections only gate the first PV — emitting
                        # them here shortens the PE-only prefix so ACT
                        # starts on exp sooner
                        for tt in range(NT):
                            v_feats(tt)
                    if pending is not None:
                        attn_pv(*pending)
                        if pending[0] == 1:
                            proj_chunk(pending[1])
                    pending = (u[0], u[1], ep)
                attn_pv(*pending)
                proj_chunk(pending[1])

    nc.compile()
    _BUILD_CACHE[key] = nc
    return nc


def _bf16(a):
    return np.ascontiguousarray(a).astype(ml_dtypes.bfloat16)


def kernel(**inputs):
    global LAST_EXEC_NS
    x = np.asarray(inputs["x"], np.float32)
    w_qkv = np.asarray(inputs["w_qkv"], np.float32)
    b_qkv = np.asarray(inputs["b_qkv"], np.float32)
    q_gamma = np.asarray(inputs["q_gamma"], np.float32)
    q_beta = np.asarray(inputs["q_beta"], np.float32)
    k_gamma = np.asarray(inputs["k_gamma"], np.float32)
    k_beta = np.asarray(inputs["k_beta"], np.float32)
    w_proj = np.asarray(inputs["w_proj"], np.float32)
    b_proj = np.asarray(inputs["b_proj"], np.float32)

    has_qgamma = not bool(np.all(q_gamma == 1.0))
    has_kgamma = not bool(np.all(k_gamma == 1.0))
    has_qbeta = bool(np.any(q_beta != 0.0))
    has_kbeta = bool(np.any(k_beta != 0.0))
    has_vbias = bool(np.any(b_qkv[2 * DIM:3 * DIM] != 0.0))
    nc = _build(has_qgamma, has_kgamma, has_qbeta, has_kbeta, has_vbias)

    # shared constants
    Cd = np.eye(D, dtype=np.float32) - 1.0 / D   # centering matrix (folded
    O2 = np.zeros((128, 2), np.float32)          # into the qkv weights below)
    O2[:D, 0] = 1.0
    O2[D:, 1] = 1.0
    gamma_cols = np.stack([np.tile(q_gamma, 2), np.tile(k_gamma, 2)],
                          axis=1).astype(np.float32)
    ones512 = np.ones((1, 512), np.float32)
    beta_cols = np.stack([np.tile(q_beta, 2), np.tile(k_beta, 2)],
                         axis=1).astype(np.float32)
    # pre-center the q/k projection weights and biases per head:
    # LN(Wx+b) centering is linear, so fold (I - J/64) into W and b
    w_qkv = w_qkv.copy()
    b_qkv = b_qkv.copy()
    for h in range(2 * H):            # 16 q heads then 16 k heads
        rs = slice(h * D, (h + 1) * D)
        w_qkv[rs] = Cd @ w_qkv[rs]
        b_qkv[rs] = Cd @ b_qkv[rs]

    in_maps = []
    for c in range(N_CORES):
        b, hg = divmod(c, 4)
        rows = slice(hg * HF, (hg + 1) * HF)
        q_l = w_qkv[0 * DIM:1 * DIM][rows]           # [256, 1024]
        k_l = w_qkv[1 * DIM:2 * DIM][rows]
        v_l = w_qkv[2 * DIM:3 * DIM][rows]
        bq_l = b_qkv[0 * DIM:1 * DIM][rows]
        bk_l = b_qkv[1 * DIM:2 * DIM][rows]
        bv_l = b_qkv[2 * DIM:3 * DIM][rows]
        bqk_cols = np.stack([bq_l[:128], bq_l[128:], bk_l[:128], bk_l[128:]],
                            axis=1).astype(np.float32)
        m = {
            "xT": _bf16(x[b].T),                          # [1024, 2048]
            "wqkT": _bf16(np.concatenate([q_l, k_l], 0).T),   # [1024, 512]
            "wvT": _bf16(v_l.T),                          # [1024, 256]
            "wpT": _bf16(w_proj[:, rows].T),              # [256, 1024]
            "bqk_cols": bqk_cols,
            "bvT": _bf16(bv_l[None, :]),
            "O2": _bf16(O2),
            "ones512": _bf16(ones512),
        }
        if has_qgamma or has_kgamma:
            m["gamma_cols"] = gamma_cols
        if has_qbeta or has_kbeta:
            m["beta_cols"] = beta_cols
        in_maps.append(m)

    res = run_bass_kernel_spmd(nc, in_maps, core_ids=list(range(N_CORES)),
                               trace=TRACE)
    LAST_EXEC_NS = res.exec_time_ns
    globals()["LAST_RESULTS"] = res

    out = np.zeros((B, N, DIM), np.float32)
    for c in range(N_CORES):
        out[c // 4] += np.asarray(res.results[c]["out_partial"], np.float32)
    out += b_proj[None, None, :]
    return out
